# revision 40
# baseline (speedup 1.0000x reference)
"""NetVLAD Trainium2 kernel (8 NeuronCores, data-parallel over points).

Strategy (v2):
  - Host: l2-normalize feat -> x (removes the whole on-device 1/||x||
    pipeline), stable-sort points by batch_id, pad each batch to
    8*128*T_b rows with ZERO rows whose mask column is 0 (pads then
    contribute exactly nothing -> no correction step). Ship x twice in
    bf16: featN [P, TT, 257] (rows + mask col, aggregation operand) and
    featT [256, NP] (logits matmul operand).
  - conv_b spans ~[-9800, -7300] and needs ~0.05 abs precision, too
    much for bf16: split b = b_hi + b_mid + b_lo (3 bf16 rows) and fold
    it into the logits PSUM with one rank-3 matmul per group (free=512).
  - Device per group of G=8 tiles (1024 points), one PSUM bank [P,512]:
    bias matmul + 16 chunk matmuls -> biased logits; one batched negated
    row-max (DVE); 8x Exp with per-partition bias (ACT, psum->sbuf bf16);
    one batched Z reduce + reciprocal; 8x soft2 = e * (1/Z); 8x
    aggregation matmuls into per-batch-pair PSUM half-banks.
  - Per completed batch pair: evac psum -> bf16, AllReduce in bf16
    (values are O(1e3) partial sums; bf16 noise is ~0.4% of terms that
    only perturb vlad scale, not direction).
  - Tail: pairs' vlad normalize + transposes overlap the last pair's
    AllReduce; FC with per-core output slice (col-packed matmuls),
    AllGather, final l2norm.
"""

import numpy as np
import ml_dtypes

BF16 = ml_dtypes.bfloat16
FP8 = ml_dtypes.float8_e4m3

N, C, K, B, OUT = 200000, 256, 64, 8, 1024
NCORES = 8
P = 128
G = 8  # tiles per group

_compiled_cache = {}
PROFILE = False       # set True to capture an NTFF profile (test harness only)
LAST_RESULT = None    # BassKernelResults of the most recent run


# ----------------------------------------------------------------------------
# Host-side planning
# ----------------------------------------------------------------------------

def _plan(feat, batch_ids):
    """Normalize rows, sort by batch, pad each batch to NCORES*P*T_b zero
    rows (mask col 0), build per-core shards."""
    nrm = np.sqrt(np.einsum("nc,nc->n", feat, feat, dtype=np.float64))
    x = feat / np.maximum(nrm, 1e-12)[:, None].astype(np.float32)

    order = np.argsort(batch_ids, kind="stable")
    x_s = x[order]
    counts = np.bincount(batch_ids, minlength=B)

    Ts = [int(np.ceil(c / (NCORES * P))) for c in counts]

    per_core = [[] for _ in range(NCORES)]
    per_core_mask = [[] for _ in range(NCORES)]
    off = 0
    for b in range(B):
        nb = int(counts[b])
        xb = x_s[off:off + nb]
        off += nb
        tot = NCORES * P * Ts[b]
        n_pad = tot - nb
        if n_pad:
            xb = np.concatenate([xb, np.zeros((n_pad, C), np.float32)], 0)
        mb = np.zeros((tot,), np.float32)
        mb[:nb] = 1.0
        xb = xb.reshape(NCORES, P * Ts[b], C)
        mb = mb.reshape(NCORES, P * Ts[b])
        for i in range(NCORES):
            per_core[i].append(xb[i])
            per_core_mask[i].append(mb[i])

    core_x = [np.concatenate(chunks, 0) for chunks in per_core]      # [NP, C]
    core_m = [np.concatenate(chunks, 0) for chunks in per_core_mask]  # [NP]
    return core_x, core_m, Ts


# ----------------------------------------------------------------------------
# Device program
# ----------------------------------------------------------------------------

def _build_nc(Ts):
    import concourse.bass as bass
    import concourse.bacc as bacc
    import concourse.mybir as mybir
    from concourse import tile

    dt = mybir.dt
    AF = mybir.ActivationFunctionType
    ALU = mybir.AluOpType

    TT = sum(Ts)            # tiles per core
    NP = TT * P             # points per core
    tile_batch = []         # batch id of each tile
    for b in range(B):
        tile_batch += [b] * Ts[b]

    nc = bacc.Bacc(
        "TRN2", target_bir_lowering=False, debug=False, num_devices=NCORES
    )

    # --- I/O ---
    featN_d = nc.dram_tensor("featN", [P, TT, C + 1], dt.bfloat16, kind="ExternalInput").ap()
    featT_d = nc.dram_tensor("featT", [C, NP], dt.float8e4, kind="ExternalInput").ap()
    wt_d = nc.dram_tensor("wt", [C, K], dt.float8e4, kind="ExternalInput").ap()
    b3_d = nc.dram_tensor("b3", [3, G * K], dt.bfloat16, kind="ExternalInput").ap()
    ones3_d = nc.dram_tensor("ones3", [3, P], dt.bfloat16, kind="ExternalInput").ap()
    cent2_d = nc.dram_tensor("cent2", [P, C], dt.bfloat16, kind="ExternalInput").ap()
    fwt_d = nc.dram_tensor("fwt", [P, K * C], dt.bfloat16, kind="ExternalInput").ap()
    fbb_d = nc.dram_tensor("fbb", [B, OUT // NCORES], dt.float32, kind="ExternalInput").ap()
    ident_d = nc.dram_tensor("ident", [P, P], dt.bfloat16, kind="ExternalInput").ap()
    sel_d = nc.dram_tensor("sel", [P, B], dt.float32, kind="ExternalInput").ap()
    out_d = nc.dram_tensor("out", [B, OUT], dt.float32, kind="ExternalOutput").ap()

    OSL = OUT // NCORES  # 128 output slice per core

    with tile.TileContext(nc) as tc:
        with (
            tc.tile_pool(name="const", bufs=1) as cpool,
            tc.tile_pool(name="dram", bufs=1, space="DRAM") as dram,
        ):
            # loop-critical consts (bias matmul operands + wt) first on the
            # gpsimd queue; tail-only consts after. The sync queue is kept
            # free to start streaming group 0's feat immediately.
            b3_sb = cpool.tile([3, G * K], dt.bfloat16, name="b3_sb")
            nc.gpsimd.dma_start(out=b3_sb[:, :], in_=b3_d[:, :])
            ones3_sb = cpool.tile([3, P], dt.bfloat16, name="ones3_sb")
            nc.gpsimd.dma_start(out=ones3_sb[:, :], in_=ones3_d[:, :])
            wt_sb = cpool.tile([P, 2, K], dt.float8e4, name="wt_sb")
            for h in range(2):
                nc.scalar.dma_start(out=wt_sb[:, h, :],
                                    in_=wt_d[h * P:(h + 1) * P, :])
            cent2_sb = cpool.tile([P, C], dt.bfloat16, name="cent2_sb")
            nc.gpsimd.dma_start(out=cent2_sb[:, :], in_=cent2_d[:, :])
            ident_sb = cpool.tile([P, P], dt.bfloat16, name="ident_sb")
            nc.gpsimd.dma_start(out=ident_sb[:, :], in_=ident_d[:, :])
            fbb_sb = cpool.tile([B, OSL], dt.float32, name="fbb_sb")
            nc.gpsimd.dma_start(out=fbb_sb[:, :], in_=fbb_d[:, :])
            # fwt (4MB, needed only for the FC tail) is DMAed mid-loop when
            # the scalar queue is idle - see the main loop below
            fwt_sb = cpool.tile([P, K * C], dt.bfloat16, name="fwt_sb")

            # vlad-phase tiles that outlive the main-loop pools
            vpool_ctx = tc.tile_pool(name="vlad", bufs=1)
            vpool = vpool_ctx.__enter__()
            vbf = [vpool.tile([P, C], dt.bfloat16, name=f"vbf{i}")
                   for i in range(4)]
            ssv = vpool.tile([P, 4], dt.float32, name="ssv")
            lnv = vpool.tile([P, 4], dt.float32, name="lnv")
            rnv = vpool.tile([P, 4], dt.float32, name="rnv")

            # ---------------- main point loop ----------------
            with (
                tc.tile_pool(name="aggp", bufs=1, space="PSUM") as aggp,
                tc.tile_pool(name="psl", bufs=3, space="PSUM") as pslp,
                tc.tile_pool(name="grp", bufs=8) as gpool,
                tc.tile_pool(name="tl", bufs=8) as tpool,
            ):
                agg = [aggp.tile([P, C + 1], dt.float32, name=f"agg{i}")
                       for i in range(4)]

                # psum evac is eager per pair; the AllReduces are batched in
                # two units (each CC op has ~12-30us latency on the serial
                # stream): pairs 0-1 at 50% of the loop, pairs 2-3 at 100%
                pair_last = [sum(Ts[:2 * p + 2]) - 1 for p in range(4)]
                pair_evaced = [False] * 4
                ev_units = [
                    ([0, 1, 2], sum(Ts[:6]) - 1),
                    ([3], sum(Ts[:8]) - 1),
                ]
                evaced = [False] * len(ev_units)

                part_d = dram.tile([B * K, C + 1], dt.bfloat16, name="part_d")
                red_d = dram.tile([B * K, C + 1], dt.bfloat16, name="red_d")

                # tiny dummy AllReduce posted first: pays the ~25us CC
                # first-op setup concurrently with the loop, so the real
                # AllReduces go through the stream at wire speed
                dmy = tpool.tile([1, 2], dt.bfloat16, name="dmy")
                nc.vector.memset(dmy[:, :], 0.0)
                dmy_in = dram.tile([1, 2], dt.bfloat16, name="dmy_in")
                dmy_out = dram.tile([1, 2], dt.bfloat16, name="dmy_out")
                nc.gpsimd.dma_start(out=dmy_in[:, :], in_=dmy[:, :])
                nc.gpsimd.collective_compute(
                    "AllReduce",
                    ALU.add,
                    replica_groups=[list(range(NCORES))],
                    ins=[dmy_in[:, :]],
                    outs=[dmy_out[:, :]],
                )

                def emit_agg(t0, gs, s2_g, featN_g):
                    for g in range(gs):
                        tt = t0 + g
                        bb_idx = tile_batch[tt]
                        pair, half = bb_idx // 2, bb_idx % 2
                        first = (tt == 0) or (tile_batch[tt - 1] != bb_idx)
                        last = (tt == TT - 1) or (tile_batch[tt + 1] != bb_idx)
                        nc.tensor.matmul(
                            agg[pair][half * K:(half + 1) * K, :],
                            lhsT=s2_g[:, g * K:(g + 1) * K],
                            rhs=featN_g[:, g, :],
                            start=first, stop=last,
                            tile_position=(0, half * K),
                        )
                    # eager evac per completed pair (bf16)
                    for pp in range(4):
                        if not pair_evaced[pp] and pair_last[pp] < t0 + gs:
                            pair_evaced[pp] = True
                            ev = tpool.tile([P, C + 1], dt.bfloat16,
                                            name="ev", tag="ev", bufs=3)
                            nc.scalar.copy(ev[:, :], agg[pp][:, :])
                            nc.gpsimd.dma_start(
                                out=part_d[pp * P:(pp + 1) * P, :],
                                in_=ev[:, :])
                    # all-reduce per completed unit
                    for u, (pairs, lt) in enumerate(ev_units):
                        if not evaced[u] and lt < t0 + gs:
                            evaced[u] = True
                            lo, hi = pairs[0] * P, (pairs[-1] + 1) * P
                            nc.gpsimd.collective_compute(
                                "AllReduce",
                                ALU.add,
                                replica_groups=[list(range(NCORES))],
                                ins=[part_d[lo:hi, :]],
                                outs=[red_d[lo:hi, :]],
                            )

                t = 0
                prev = None
                prev2 = None
                while t < TT:
                    gs = min(G, TT - t)
                    featT_g = gpool.tile([P, 2, G * P], dt.float8e4, name="featT_g")
                    featN_g = gpool.tile([P, G, C + 1], dt.bfloat16, name="featN_g")
                    negm_g = gpool.tile([P, G], dt.float32, name="negm_g")
                    z_g = gpool.tile([P, G], dt.float32, name="z_g")
                    rz_g = gpool.tile([P, G], dt.float32, name="rz_g")
                    e_g = gpool.tile([P, G * K], dt.bfloat16, name="e_g")
                    s2_g = gpool.tile([P, G * K], dt.bfloat16, name="s2_g")

                    # featT: two [128, gs*128] contiguous slabs (c-halves) on
                    # the sync queue; featN on the gpsimd queue; group 0 on
                    # the scalar queue (its preamble finishes first)
                    featT_q = nc.scalar if t == 0 else nc.sync
                    featN_q = nc.scalar if t == 0 else nc.gpsimd
                    for h in range(2):
                        featT_q.dma_start(
                            out=featT_g[:, h, 0:gs * P],
                            in_=featT_d[h * P:(h + 1) * P, t * P:(t + gs) * P],
                        )
                    # flat views -> one contiguous 4KB descriptor/partition
                    CW = C + 1
                    featN_q.dma_start(
                        out=featN_g.rearrange("p g c -> p (g c)")[
                            :, 0:gs * CW],
                        in_=featN_d.rearrange("p t c -> p (t c)")[
                            :, t * CW:(t + gs) * CW],
                    )

                    # biased logits for the whole group in one PSUM bank:
                    # bias via rank-3 matmul (b split into 3 bf16 rows)
                    psumL = pslp.tile([P, G * K], dt.float32, name="psumL")
                    nc.tensor.matmul(
                        psumL[:, 0:gs * K],
                        lhsT=ones3_sb[:, :],
                        rhs=b3_sb[:, 0:gs * K],
                        start=True, stop=False,
                        skip_group_check=True,
                    )
                    for g in range(gs):
                        for h in range(2):
                            nc.tensor.matmul(
                                psumL[:, (g * K):(g + 1) * K],
                                lhsT=featT_g[:, h, g * P:(g + 1) * P],
                                rhs=wt_sb[:, h, :],
                                start=False, stop=(h == 1),
                                skip_group_check=True,
                            )

                    # aggregation matmuls run two groups behind the logits so
                    # the PE never waits on the DVE/ACT softmax chain
                    if prev2 is not None:
                        emit_agg(*prev2)

                    # batched negated row max over the whole group
                    nc.vector.tensor_reduce(
                        out=negm_g[:, 0:gs],
                        in_=psumL.rearrange("p (g k) -> p g k", k=K)[:, 0:gs, :],
                        axis=mybir.AxisListType.X,
                        op=ALU.max,
                        negate=True,
                    )
                    # e = exp(t3 - m), psum -> sbuf bf16
                    for g in range(gs):
                        nc.scalar.activation(
                            e_g[:, g * K:(g + 1) * K],
                            psumL[:, g * K:(g + 1) * K],
                            AF.Exp,
                            bias=negm_g[:, g:g + 1],
                        )
                    # batched Z = sum_k e
                    nc.vector.tensor_reduce(
                        out=z_g[:, 0:gs],
                        in_=e_g.rearrange("p (g k) -> p g k", k=K)[:, 0:gs, :],
                        axis=mybir.AxisListType.X,
                        op=ALU.add,
                    )
                    nc.vector.reciprocal(rz_g[:, 0:gs], z_g[:, 0:gs])

                    # soft2 = e * (1/Z) in ONE batched DVE op: rz broadcast
                    # along k via a zero-stride AP
                    e_view = e_g.rearrange("p (g k) -> p g k", k=K)[:, 0:gs, :]
                    s2_view = s2_g.rearrange("p (g k) -> p g k", k=K)[:, 0:gs, :]
                    rz3 = rz_g.rearrange("p (g one) -> p g one", one=1)[:, 0:gs, :]
                    rz_bcast, e_bcast = bass.broadcast_tensor_aps(rz3, e_view)
                    nc.vector.tensor_tensor(
                        out=s2_view,
                        in0=e_bcast,
                        in1=rz_bcast,
                        op=ALU.mult,
                    )
                    prev2 = prev
                    prev = (t, gs, s2_g, featN_g)
                    t += gs
                # FC weights (4MB) stream in right after the loop's last
                # DMAs - the HBM is idle then, and they arrive before the
                # FC needs them (during the tail AllReduces)
                for q in range(4):
                    qs = K * C // 4
                    nc.scalar.dma_start(
                        out=fwt_sb[:, q * qs:(q + 1) * qs],
                        in_=fwt_d[:, q * qs:(q + 1) * qs])
                if prev2 is not None:
                    emit_agg(*prev2)
                emit_agg(*prev)
                assert all(evaced)

            # ---------------- vlad + fc ----------------
            with (
                tc.tile_pool(name="fin", bufs=1) as fpool,
                tc.tile_pool(name="fps", bufs=2, space="PSUM") as fpsum,
                tc.tile_pool(name="fcp", bufs=1, space="PSUM") as fcps,
            ):
                vT = [fpool.tile([P, 4 * P], dt.bfloat16, name=f"vT{h}")
                      for h in range(2)]
                for i in range(4):
                    ared = fpool.tile([P, C + 1], dt.bfloat16, name="ared",
                                      tag="ared", bufs=2)
                    nc.sync.dma_start(out=ared[:, :],
                                      in_=red_d[i * P:(i + 1) * P, :])
                    # nv = cent*S - A   (negated vlad; fc weights negated)
                    nv = fpool.tile([P, C], dt.float32, name="nv", tag="nv",
                                    bufs=2)
                    nc.vector.scalar_tensor_tensor(
                        out=nv[:, :], in0=cent2_sb[:, :],
                        scalar=ared[:, C:C + 1], in1=ared[:, 0:C],
                        op0=ALU.mult, op1=ALU.subtract)
                    nvs = fpool.tile([P, C], dt.float32, name="nvs", tag="nvs",
                                     bufs=2)
                    nc.vector.scalar_tensor_tensor(
                        out=nvs[:, :], in0=nv[:, :], scalar=1.0, in1=nv[:, :],
                        op0=ALU.mult, op1=ALU.mult,
                        accum_out=ssv[:, i:i + 1])
                    nc.vector.tensor_scalar_max(
                        ssv[:, i:i + 1], ssv[:, i:i + 1], 1e-24)
                    nc.scalar.activation(lnv[:, i:i + 1], ssv[:, i:i + 1], AF.Ln)
                    nc.scalar.activation(rnv[:, i:i + 1], lnv[:, i:i + 1],
                                         AF.Exp, scale=-0.5)
                    nc.vector.tensor_scalar(
                        out=vbf[i][:, :], in0=nv[:, :],
                        scalar1=rnv[:, i:i + 1], scalar2=None, op0=ALU.mult)
                    # transpose the two c-halves into vT buffers
                    for h in range(2):
                        pt = fpsum.tile([P, P], dt.bfloat16, name="pt")
                        nc.tensor.transpose(
                            pt[:, :], vbf[i][:, h * P:(h + 1) * P], ident_sb[:, :])
                        nc.vector.tensor_copy(
                            vT[h][:, i * P:(i + 1) * P], pt[:, :])

                # FC: out[8b, 128o] in 4 concurrent col-groups, separate banks
                NCH = K * C // P  # 128 contraction chunks
                vTv = [vT[h].rearrange("p (b k) -> p k b", b=B) for h in range(2)]
                fcpg = [fcps.tile([P, OSL], dt.float32, name=f"fcp{g}", bufs=1)
                        for g in range(4)]
                for j in range(NCH):
                    grp = j % 4
                    lhsT = vTv[j % 2][:, j // 2, :]  # [128, 8] strided cols
                    nc.tensor.matmul(
                        fcpg[grp][32 * grp:32 * grp + B, :],
                        lhsT=lhsT,
                        rhs=fwt_sb[:, j * OSL:(j + 1) * OSL],
                        start=(j < 4), stop=(j >= NCH - 4),
                        tile_position=(0, 32 * grp),
                        skip_group_check=True,
                    )
                # gather the 4 partition-offset groups into one [128, OSL]
                # SBUF tile, then sum across partitions with a selector matmul
                sb4 = fpool.tile([P, OSL], dt.float32, name="sb4")
                nc.vector.memset(sb4[:, :], 0.0)
                for g in range(4):
                    nc.scalar.copy(
                        sb4[32 * g:32 * g + B, :],
                        fcpg[g][32 * g:32 * g + B, :])
                sel_sb = cpool.tile([P, B], dt.float32, name="sel_sb")
                nc.sync.dma_start(out=sel_sb[:, :], in_=sel_d[:, :])
                fcsum = fcps.tile([P, OSL], dt.float32, name="fcsum", bufs=1)
                nc.tensor.matmul(
                    fcsum[0:B, :], lhsT=sel_sb[:, :], rhs=sb4[:, :],
                    start=True, stop=True, skip_group_check=True,
                )
                fo = fpool.tile([B, OSL], dt.float32, name="fo")
                nc.vector.tensor_tensor(
                    out=fo[:, :], in0=fcsum[0:B, :], in1=fbb_sb[:, :],
                    op=ALU.add)

                # AllGather the [8, 128] slices
                ag_in = dram.tile([B, OSL], dt.float32, name="ag_in")
                ag_out = dram.tile([NCORES * B, OSL], dt.float32, name="ag_out")
                nc.sync.dma_start(out=ag_in[:, :], in_=fo[:, :])
                nc.gpsimd.collective_compute(
                    "AllGather",
                    ALU.bypass,
                    replica_groups=[list(range(NCORES))],
                    ins=[ag_in[:, :]],
                    outs=[ag_out[:, :]],
                )
                # reassemble [8, 1024], then final l2norm computed in place
                fin = fpool.tile([B, OUT], dt.float32, name="fin")
                agv = ag_out.rearrange("(c b) o -> b c o", b=B)
                nc.sync.dma_start(
                    out=fin.rearrange("b (c o) -> b c o", c=NCORES),
                    in_=agv[:, :, :],
                )
                fsc = fpool.tile([B, OUT], dt.float32, name="fsc")
                ssf = fpool.tile([B, 1], dt.float32, name="ssf")
                lnf = fpool.tile([B, 1], dt.float32, name="lnf")
                rnf = fpool.tile([B, 1], dt.float32, name="rnf")
                nc.vector.scalar_tensor_tensor(
                    out=fsc[:, :], in0=fin[:, :], scalar=1.0,
                    in1=fin[:, :], op0=ALU.mult, op1=ALU.mult,
                    accum_out=ssf[:, 0:1])
                nc.vector.tensor_scalar_max(ssf[:, :], ssf[:, :], 1e-24)
                nc.scalar.activation(lnf[:, :], ssf[:, :], AF.Ln)
                nc.scalar.activation(rnf[:, :], lnf[:, :], AF.Exp, scale=-0.5)
                fout = fpool.tile([B, OUT], dt.float32, name="fout")
                nc.vector.tensor_scalar(
                    out=fout[:, :], in0=fin[:, :],
                    scalar1=rnf[:, 0:1], scalar2=None, op0=ALU.mult)
                nc.sync.dma_start(out=out_d[:, :], in_=fout[:, :])

            vpool_ctx.__exit__(None, None, None)

    # Force every activation onto the one table set that holds Exp+Ln
    # (+Copy/Identity) together -- the default per-function choice thrashes
    # ACT_TABLE_LOADs (~1.3us each) between exp_and_others / natural_log.
    import types
    import bass_rust as _bass_rust
    from concourse.hw_specs import get_activation_tables

    def _act_tables_one_set(self):
        has_activation = any(
            isinstance(i, mybir.InstActivation)
            for b in self.main_func.blocks
            for i in b.instructions
        )
        if not has_activation:
            return
        tables = get_activation_tables(self.m.arch)
        pref = "natural_log_exp_and_others"
        mod = [(k, (v if k == pref else set())) for k, v in tables.items()]
        _bass_rust.insert_act_table_loads(self, mod)

    nc.insert_act_table_loads = types.MethodType(_act_tables_one_set, nc)

    nc.compile()
    return nc


# ----------------------------------------------------------------------------
# Host-side input assembly per core
# ----------------------------------------------------------------------------

def _make_in_maps(feat, batch_ids, conv_w, conv_b, centroids, fc_w, fc_b):
    core_x, core_m, Ts = _plan(feat, batch_ids)

    wt = np.ascontiguousarray(conv_w.T / 8.0).astype(FP8)      # [256, 64]
    # conv_b split into 3 bf16 rows (exact to ~1e-4 abs), tiled G times
    b_hi = conv_b.astype(BF16)
    r1 = conv_b - b_hi.astype(np.float32)
    b_mid = r1.astype(BF16)
    r2 = r1 - b_mid.astype(np.float32)
    b_lo = r2.astype(BF16)
    b3 = np.stack([b_hi, b_mid, b_lo], 0)                       # [3, 64]
    b3t = np.tile(b3, (1, G))                                   # [3, 512]
    ones3 = np.ones((3, P), np.float32).astype(BF16)

    cent2 = np.concatenate([centroids, centroids], 0).astype(BF16)  # [128, 256]
    ident = np.eye(P, dtype=np.float32).astype(BF16)
    sel = np.zeros((P, B), np.float32)
    for g in range(4):
        for b in range(B):
            sel[32 * g + b, b] = 1.0

    OSL = OUT // NCORES
    in_maps = []
    for i in range(NCORES):
        cx = core_x[i]
        cm = core_m[i]
        nt = cx.shape[0] // P
        featN = np.empty((P, nt, C + 1), dtype=BF16)
        featN[:, :, 0:C] = cx.reshape(nt, P, C).transpose(1, 0, 2).astype(BF16)
        featN[:, :, C] = cm.reshape(nt, P).T.astype(BF16)
        featT = np.ascontiguousarray(cx.T * 8.0).astype(FP8)
        # fc slice, negated (vlad computed negated), chunk-major pre-swizzle:
        # fwt_sb[p, j*128+o] = -fc_w[o_base+o, j*128+p]
        fsl = -fc_w[i * OSL:(i + 1) * OSL]                      # [128, 16384]
        fsw = np.ascontiguousarray(
            fsl.reshape(OSL, K * C // P, P).transpose(2, 1, 0).reshape(P, K * C)
        ).astype(BF16)
        fbb = np.broadcast_to(fc_b[i * OSL:(i + 1) * OSL].astype(np.float32),
                              (B, OSL)).copy()
        in_maps.append({
            "featN": featN,
            "featT": featT,
            "wt": wt,
            "b3": b3t,
            "ones3": ones3,
            "cent2": cent2,
            "fwt": fsw,
            "fbb": fbb,
            "ident": ident,
            "sel": sel,
        })
    return in_maps, Ts


def _ensure_profile_hook():
    """The agent image's `antenv` lacks `axon_hooks`; synthesize it so
    run_bass_kernel_spmd(trace=True) can reach the NTFF profiler."""
    import sys
    import types
    try:
        from antenv.axon_hooks import get_axon_ntff_profile_hook  # noqa: F401
        return True
    except ImportError:
        pass
    try:
        from trn_agent_boot.trn_boot import _ntff_profile_via_ctypes
        hook = _ntff_profile_via_ctypes("/opt/axon/libaxon_pjrt.so")
        if hook is None:
            return False
        mod = types.ModuleType("antenv.axon_hooks")
        mod._hook = hook
        mod.get_axon_ntff_profile_hook = lambda: mod._hook
        mod.set_axon_ntff_profile_hook = lambda h: setattr(mod, "_hook", h)
        import antenv
        antenv.axon_hooks = mod
        sys.modules["antenv.axon_hooks"] = mod
        return True
    except Exception:
        return False


def kernel(feat, batch_ids, centroids, conv_w, conv_b, fc_w, fc_b, batch_size):
    from concourse.bass_utils import run_bass_kernel_spmd

    feat = np.asarray(feat, dtype=np.float32)
    batch_ids = np.asarray(batch_ids, dtype=np.int32)
    centroids = np.asarray(centroids, dtype=np.float32)
    conv_w = np.asarray(conv_w, dtype=np.float32)
    conv_b = np.asarray(conv_b, dtype=np.float32)
    fc_w = np.asarray(fc_w, dtype=np.float32)
    fc_b = np.asarray(fc_b, dtype=np.float32)

    in_maps, Ts = _make_in_maps(
        feat, batch_ids, conv_w, conv_b, centroids, fc_w, fc_b)

    key = tuple(Ts)
    if key not in _compiled_cache:
        _compiled_cache[key] = _build_nc(Ts)
    nc = _compiled_cache[key]

    global LAST_RESULT
    do_trace = PROFILE and _ensure_profile_hook()
    res = run_bass_kernel_spmd(
        nc, in_maps, core_ids=list(range(NCORES)), trace=do_trace)
    LAST_RESULT = res
    return np.asarray(res.results[0]["out"], dtype=np.float32)


# revision 41
# speedup vs baseline: 1.0456x; 1.0456x over previous
"""NetVLAD Trainium2 kernel (8 NeuronCores, data-parallel over points).

Strategy (v2):
  - Host: l2-normalize feat -> x (removes the whole on-device 1/||x||
    pipeline), stable-sort points by batch_id, pad each batch to
    8*128*T_b rows with ZERO rows whose mask column is 0 (pads then
    contribute exactly nothing -> no correction step). Ship x twice in
    bf16: featN [P, TT, 257] (rows + mask col, aggregation operand) and
    featT [256, NP] (logits matmul operand).
  - conv_b spans ~[-9800, -7300] and needs ~0.05 abs precision, too
    much for bf16: split b = b_hi + b_mid + b_lo (3 bf16 rows) and fold
    it into the logits PSUM with one rank-3 matmul per group (free=512).
  - Device per group of G=8 tiles (1024 points), one PSUM bank [P,512]:
    bias matmul + 16 chunk matmuls -> biased logits; one batched negated
    row-max (DVE); 8x Exp with per-partition bias (ACT, psum->sbuf bf16);
    one batched Z reduce + reciprocal; 8x soft2 = e * (1/Z); 8x
    aggregation matmuls into per-batch-pair PSUM half-banks.
  - Per completed batch pair: evac psum -> bf16, AllReduce in bf16
    (values are O(1e3) partial sums; bf16 noise is ~0.4% of terms that
    only perturb vlad scale, not direction).
  - Tail: pairs' vlad normalize + transposes overlap the last pair's
    AllReduce; FC with per-core output slice (col-packed matmuls),
    AllGather, final l2norm.
"""

import numpy as np
import ml_dtypes

BF16 = ml_dtypes.bfloat16
FP8 = ml_dtypes.float8_e4m3

N, C, K, B, OUT = 200000, 256, 64, 8, 1024
NCORES = 8
P = 128
G = 8  # tiles per group

_compiled_cache = {}
PROFILE = False       # set True to capture an NTFF profile (test harness only)
LAST_RESULT = None    # BassKernelResults of the most recent run


# ----------------------------------------------------------------------------
# Host-side planning
# ----------------------------------------------------------------------------

def _plan(feat, batch_ids):
    """Normalize rows, sort by batch, pad each batch to NCORES*P*T_b zero
    rows (mask col 0), build per-core shards."""
    nrm = np.sqrt(np.einsum("nc,nc->n", feat, feat, dtype=np.float64))
    x = feat / np.maximum(nrm, 1e-12)[:, None].astype(np.float32)

    order = np.argsort(batch_ids, kind="stable")
    x_s = x[order]
    counts = np.bincount(batch_ids, minlength=B)

    Ts = [int(np.ceil(c / (NCORES * P))) for c in counts]

    per_core = [[] for _ in range(NCORES)]
    per_core_mask = [[] for _ in range(NCORES)]
    off = 0
    for b in range(B):
        nb = int(counts[b])
        xb = x_s[off:off + nb]
        off += nb
        tot = NCORES * P * Ts[b]
        n_pad = tot - nb
        if n_pad:
            xb = np.concatenate([xb, np.zeros((n_pad, C), np.float32)], 0)
        mb = np.zeros((tot,), np.float32)
        mb[:nb] = 1.0
        xb = xb.reshape(NCORES, P * Ts[b], C)
        mb = mb.reshape(NCORES, P * Ts[b])
        for i in range(NCORES):
            per_core[i].append(xb[i])
            per_core_mask[i].append(mb[i])

    core_x = [np.concatenate(chunks, 0) for chunks in per_core]      # [NP, C]
    core_m = [np.concatenate(chunks, 0) for chunks in per_core_mask]  # [NP]
    return core_x, core_m, Ts


# ----------------------------------------------------------------------------
# Device program
# ----------------------------------------------------------------------------

def _build_nc(Ts):
    import concourse.bass as bass
    import concourse.bacc as bacc
    import concourse.mybir as mybir
    from concourse import tile

    dt = mybir.dt
    AF = mybir.ActivationFunctionType
    ALU = mybir.AluOpType

    TT = sum(Ts)            # tiles per core
    NP = TT * P             # points per core
    tile_batch = []         # batch id of each tile
    for b in range(B):
        tile_batch += [b] * Ts[b]

    nc = bacc.Bacc(
        "TRN2", target_bir_lowering=False, debug=False, num_devices=NCORES
    )

    # --- I/O ---
    featN_d = nc.dram_tensor("featN", [P, TT, C + 1], dt.bfloat16, kind="ExternalInput").ap()
    featT_d = nc.dram_tensor("featT", [C, NP], dt.float8e4, kind="ExternalInput").ap()
    wt_d = nc.dram_tensor("wt", [C, K], dt.float8e4, kind="ExternalInput").ap()
    b3_d = nc.dram_tensor("b3", [3, G * K], dt.bfloat16, kind="ExternalInput").ap()
    ones3_d = nc.dram_tensor("ones3", [3, P], dt.bfloat16, kind="ExternalInput").ap()
    cent2_d = nc.dram_tensor("cent2", [P, C], dt.bfloat16, kind="ExternalInput").ap()
    fwt_d = nc.dram_tensor("fwt", [P, K * C], dt.bfloat16, kind="ExternalInput").ap()
    fbb_d = nc.dram_tensor("fbb", [B, OUT // NCORES], dt.float32, kind="ExternalInput").ap()
    ident_d = nc.dram_tensor("ident", [P, P], dt.bfloat16, kind="ExternalInput").ap()
    sel_d = nc.dram_tensor("sel", [P, B], dt.float32, kind="ExternalInput").ap()
    out_d = nc.dram_tensor("out", [B, OUT], dt.float32, kind="ExternalOutput").ap()

    OSL = OUT // NCORES  # 128 output slice per core

    with tile.TileContext(nc) as tc:
        with (
            tc.tile_pool(name="const", bufs=1) as cpool,
            tc.tile_pool(name="dram", bufs=1, space="DRAM") as dram,
        ):
            # loop-critical consts (bias matmul operands + wt) first on the
            # gpsimd queue; tail-only consts after. The sync queue is kept
            # free to start streaming group 0's feat immediately.
            b3_sb = cpool.tile([3, G * K], dt.bfloat16, name="b3_sb")
            nc.gpsimd.dma_start(out=b3_sb[:, :], in_=b3_d[:, :])
            ones3_sb = cpool.tile([3, P], dt.bfloat16, name="ones3_sb")
            nc.gpsimd.dma_start(out=ones3_sb[:, :], in_=ones3_d[:, :])
            wt_sb = cpool.tile([P, 2, K], dt.float8e4, name="wt_sb")
            for h in range(2):
                nc.scalar.dma_start(out=wt_sb[:, h, :],
                                    in_=wt_d[h * P:(h + 1) * P, :])
            cent2_sb = cpool.tile([P, C], dt.bfloat16, name="cent2_sb")
            nc.gpsimd.dma_start(out=cent2_sb[:, :], in_=cent2_d[:, :])
            ident_sb = cpool.tile([P, P], dt.bfloat16, name="ident_sb")
            nc.gpsimd.dma_start(out=ident_sb[:, :], in_=ident_d[:, :])
            fbb_sb = cpool.tile([B, OSL], dt.float32, name="fbb_sb")
            nc.gpsimd.dma_start(out=fbb_sb[:, :], in_=fbb_d[:, :])
            # fwt (4MB, needed only for the FC tail) is DMAed mid-loop when
            # the scalar queue is idle - see the main loop below
            fwt_sb = cpool.tile([P, K * C], dt.bfloat16, name="fwt_sb")

            # vlad-phase tiles that outlive the main-loop pools
            vpool_ctx = tc.tile_pool(name="vlad", bufs=1)
            vpool = vpool_ctx.__enter__()
            vbf = [vpool.tile([P, C], dt.bfloat16, name=f"vbf{i}")
                   for i in range(4)]
            ssv = vpool.tile([P, 4], dt.float32, name="ssv")
            lnv = vpool.tile([P, 4], dt.float32, name="lnv")
            rnv = vpool.tile([P, 4], dt.float32, name="rnv")

            # ---------------- main point loop ----------------
            with (
                tc.tile_pool(name="aggp", bufs=1, space="PSUM") as aggp,
                tc.tile_pool(name="psl", bufs=3, space="PSUM") as pslp,
                tc.tile_pool(name="grp", bufs=8) as gpool,
                tc.tile_pool(name="tl", bufs=8) as tpool,
            ):
                agg = [aggp.tile([P, C + 1], dt.float32, name=f"agg{i}")
                       for i in range(4)]

                # psum evac is eager per pair; the AllReduces are batched in
                # two units (each CC op has ~12-30us latency on the serial
                # stream): pairs 0-1 at 50% of the loop, pairs 2-3 at 100%
                pair_last = [sum(Ts[:2 * p + 2]) - 1 for p in range(4)]
                pair_evaced = [False] * 4
                ev_units = [
                    ([0, 1], sum(Ts[:4]) - 1),
                    ([2, 3], sum(Ts[:8]) - 1),
                ]
                evaced = [False] * len(ev_units)

                part_d = dram.tile([B * K, C + 1], dt.bfloat16, name="part_d")
                red_d = dram.tile([B * K, C + 1], dt.bfloat16, name="red_d")

                # tiny dummy AllReduce posted first: pays the ~25us CC
                # first-op setup concurrently with the loop, so the real
                # AllReduces go through the stream at wire speed
                dmy = tpool.tile([1, 2], dt.bfloat16, name="dmy")
                nc.vector.memset(dmy[:, :], 0.0)
                dmy_in = dram.tile([1, 2], dt.bfloat16, name="dmy_in")
                dmy_out = dram.tile([1, 2], dt.bfloat16, name="dmy_out")
                nc.gpsimd.dma_start(out=dmy_in[:, :], in_=dmy[:, :])
                nc.gpsimd.collective_compute(
                    "AllReduce",
                    ALU.add,
                    replica_groups=[list(range(NCORES))],
                    ins=[dmy_in[:, :]],
                    outs=[dmy_out[:, :]],
                )

                def emit_agg(t0, gs, s2_g, featN_g):
                    for g in range(gs):
                        tt = t0 + g
                        bb_idx = tile_batch[tt]
                        pair, half = bb_idx // 2, bb_idx % 2
                        first = (tt == 0) or (tile_batch[tt - 1] != bb_idx)
                        last = (tt == TT - 1) or (tile_batch[tt + 1] != bb_idx)
                        nc.tensor.matmul(
                            agg[pair][half * K:(half + 1) * K, :],
                            lhsT=s2_g[:, g * K:(g + 1) * K],
                            rhs=featN_g[:, g, :],
                            start=first, stop=last,
                            tile_position=(0, half * K),
                        )
                    # eager evac per completed pair (bf16)
                    for pp in range(4):
                        if not pair_evaced[pp] and pair_last[pp] < t0 + gs:
                            pair_evaced[pp] = True
                            ev = tpool.tile([P, C + 1], dt.bfloat16,
                                            name="ev", tag="ev", bufs=3)
                            nc.scalar.copy(ev[:, :], agg[pp][:, :])
                            nc.gpsimd.dma_start(
                                out=part_d[pp * P:(pp + 1) * P, :],
                                in_=ev[:, :])
                    # all-reduce per completed unit
                    for u, (pairs, lt) in enumerate(ev_units):
                        if not evaced[u] and lt < t0 + gs:
                            evaced[u] = True
                            lo, hi = pairs[0] * P, (pairs[-1] + 1) * P
                            nc.gpsimd.collective_compute(
                                "AllReduce",
                                ALU.add,
                                replica_groups=[list(range(NCORES))],
                                ins=[part_d[lo:hi, :]],
                                outs=[red_d[lo:hi, :]],
                            )

                t = 0
                prev = None
                prev2 = None
                while t < TT:
                    gs = min(G, TT - t)
                    featT_g = gpool.tile([P, 2, G * P], dt.float8e4, name="featT_g")
                    featN_g = gpool.tile([P, G, C + 1], dt.bfloat16, name="featN_g")
                    negm_g = gpool.tile([P, G], dt.float32, name="negm_g")
                    z_g = gpool.tile([P, G], dt.float32, name="z_g")
                    rz_g = gpool.tile([P, G], dt.float32, name="rz_g")
                    e_g = gpool.tile([P, G * K], dt.bfloat16, name="e_g")
                    s2_g = gpool.tile([P, G * K], dt.bfloat16, name="s2_g")

                    # featT: two [128, gs*128] contiguous slabs (c-halves) on
                    # the sync queue; featN on the gpsimd queue; group 0 on
                    # the scalar queue (its preamble finishes first)
                    featT_q = nc.scalar if t == 0 else nc.sync
                    featN_q = nc.scalar if t == 0 else nc.gpsimd
                    for h in range(2):
                        featT_q.dma_start(
                            out=featT_g[:, h, 0:gs * P],
                            in_=featT_d[h * P:(h + 1) * P, t * P:(t + gs) * P],
                        )
                    # flat views -> one contiguous 4KB descriptor/partition
                    CW = C + 1
                    featN_q.dma_start(
                        out=featN_g.rearrange("p g c -> p (g c)")[
                            :, 0:gs * CW],
                        in_=featN_d.rearrange("p t c -> p (t c)")[
                            :, t * CW:(t + gs) * CW],
                    )

                    # biased logits for the whole group in one PSUM bank:
                    # bias via rank-3 matmul (b split into 3 bf16 rows)
                    psumL = pslp.tile([P, G * K], dt.float32, name="psumL")
                    nc.tensor.matmul(
                        psumL[:, 0:gs * K],
                        lhsT=ones3_sb[:, :],
                        rhs=b3_sb[:, 0:gs * K],
                        start=True, stop=False,
                        skip_group_check=True,
                    )
                    for g in range(gs):
                        for h in range(2):
                            nc.tensor.matmul(
                                psumL[:, (g * K):(g + 1) * K],
                                lhsT=featT_g[:, h, g * P:(g + 1) * P],
                                rhs=wt_sb[:, h, :],
                                start=False, stop=(h == 1),
                                skip_group_check=True,
                            )

                    # aggregation matmuls run two groups behind the logits so
                    # the PE never waits on the DVE/ACT softmax chain
                    if prev2 is not None:
                        emit_agg(*prev2)

                    # batched negated row max over the whole group
                    nc.vector.tensor_reduce(
                        out=negm_g[:, 0:gs],
                        in_=psumL.rearrange("p (g k) -> p g k", k=K)[:, 0:gs, :],
                        axis=mybir.AxisListType.X,
                        op=ALU.max,
                        negate=True,
                    )
                    # e = exp(t3 - m), psum -> sbuf bf16
                    for g in range(gs):
                        nc.scalar.activation(
                            e_g[:, g * K:(g + 1) * K],
                            psumL[:, g * K:(g + 1) * K],
                            AF.Exp,
                            bias=negm_g[:, g:g + 1],
                        )
                    # batched Z = sum_k e
                    nc.vector.tensor_reduce(
                        out=z_g[:, 0:gs],
                        in_=e_g.rearrange("p (g k) -> p g k", k=K)[:, 0:gs, :],
                        axis=mybir.AxisListType.X,
                        op=ALU.add,
                    )
                    nc.vector.reciprocal(rz_g[:, 0:gs], z_g[:, 0:gs])

                    # soft2 = e * (1/Z) in ONE batched DVE op: rz broadcast
                    # along k via a zero-stride AP
                    e_view = e_g.rearrange("p (g k) -> p g k", k=K)[:, 0:gs, :]
                    s2_view = s2_g.rearrange("p (g k) -> p g k", k=K)[:, 0:gs, :]
                    rz3 = rz_g.rearrange("p (g one) -> p g one", one=1)[:, 0:gs, :]
                    rz_bcast, e_bcast = bass.broadcast_tensor_aps(rz3, e_view)
                    nc.vector.tensor_tensor(
                        out=s2_view,
                        in0=e_bcast,
                        in1=rz_bcast,
                        op=ALU.mult,
                    )
                    prev2 = prev
                    prev = (t, gs, s2_g, featN_g)
                    t += gs
                # FC weights (4MB) stream in right after the loop's last
                # DMAs - the HBM is idle then, and they arrive before the
                # FC needs them (during the tail AllReduces)
                for q in range(4):
                    qs = K * C // 4
                    nc.scalar.dma_start(
                        out=fwt_sb[:, q * qs:(q + 1) * qs],
                        in_=fwt_d[:, q * qs:(q + 1) * qs])
                if prev2 is not None:
                    emit_agg(*prev2)
                emit_agg(*prev)
                assert all(evaced)

            # ---------------- vlad + fc ----------------
            with (
                tc.tile_pool(name="fin", bufs=1) as fpool,
                tc.tile_pool(name="fps", bufs=2, space="PSUM") as fpsum,
                tc.tile_pool(name="fcp", bufs=1, space="PSUM") as fcps,
            ):
                vT = [fpool.tile([P, 4 * P], dt.bfloat16, name=f"vT{h}")
                      for h in range(2)]
                for i in range(4):
                    ared = fpool.tile([P, C + 1], dt.bfloat16, name="ared",
                                      tag="ared", bufs=2)
                    nc.sync.dma_start(out=ared[:, :],
                                      in_=red_d[i * P:(i + 1) * P, :])
                    # nv = cent*S - A   (negated vlad; fc weights negated)
                    nv = fpool.tile([P, C], dt.float32, name="nv", tag="nv",
                                    bufs=2)
                    nc.vector.scalar_tensor_tensor(
                        out=nv[:, :], in0=cent2_sb[:, :],
                        scalar=ared[:, C:C + 1], in1=ared[:, 0:C],
                        op0=ALU.mult, op1=ALU.subtract)
                    nvs = fpool.tile([P, C], dt.float32, name="nvs", tag="nvs",
                                     bufs=2)
                    nc.vector.scalar_tensor_tensor(
                        out=nvs[:, :], in0=nv[:, :], scalar=1.0, in1=nv[:, :],
                        op0=ALU.mult, op1=ALU.mult,
                        accum_out=ssv[:, i:i + 1])
                    nc.vector.tensor_scalar_max(
                        ssv[:, i:i + 1], ssv[:, i:i + 1], 1e-24)
                    nc.scalar.activation(lnv[:, i:i + 1], ssv[:, i:i + 1], AF.Ln)
                    nc.scalar.activation(rnv[:, i:i + 1], lnv[:, i:i + 1],
                                         AF.Exp, scale=-0.5)
                    nc.vector.tensor_scalar(
                        out=vbf[i][:, :], in0=nv[:, :],
                        scalar1=rnv[:, i:i + 1], scalar2=None, op0=ALU.mult)
                    # transpose the two c-halves into vT buffers
                    for h in range(2):
                        pt = fpsum.tile([P, P], dt.bfloat16, name="pt")
                        nc.tensor.transpose(
                            pt[:, :], vbf[i][:, h * P:(h + 1) * P], ident_sb[:, :])
                        nc.vector.tensor_copy(
                            vT[h][:, i * P:(i + 1) * P], pt[:, :])

                # FC: out[8b, 128o] in 4 concurrent col-groups, separate banks
                NCH = K * C // P  # 128 contraction chunks
                vTv = [vT[h].rearrange("p (b k) -> p k b", b=B) for h in range(2)]
                fcpg = [fcps.tile([P, OSL], dt.float32, name=f"fcp{g}", bufs=1)
                        for g in range(4)]
                for j in range(NCH):
                    grp = j % 4
                    lhsT = vTv[j % 2][:, j // 2, :]  # [128, 8] strided cols
                    nc.tensor.matmul(
                        fcpg[grp][32 * grp:32 * grp + B, :],
                        lhsT=lhsT,
                        rhs=fwt_sb[:, j * OSL:(j + 1) * OSL],
                        start=(j < 4), stop=(j >= NCH - 4),
                        tile_position=(0, 32 * grp),
                        skip_group_check=True,
                    )
                # gather the 4 partition-offset groups into one [128, OSL]
                # SBUF tile, then sum across partitions with a selector matmul
                sb4 = fpool.tile([P, OSL], dt.float32, name="sb4")
                nc.vector.memset(sb4[:, :], 0.0)
                for g in range(4):
                    nc.scalar.copy(
                        sb4[32 * g:32 * g + B, :],
                        fcpg[g][32 * g:32 * g + B, :])
                sel_sb = cpool.tile([P, B], dt.float32, name="sel_sb")
                nc.sync.dma_start(out=sel_sb[:, :], in_=sel_d[:, :])
                fcsum = fcps.tile([P, OSL], dt.float32, name="fcsum", bufs=1)
                nc.tensor.matmul(
                    fcsum[0:B, :], lhsT=sel_sb[:, :], rhs=sb4[:, :],
                    start=True, stop=True, skip_group_check=True,
                )
                fo = fpool.tile([B, OSL], dt.float32, name="fo")
                nc.vector.tensor_tensor(
                    out=fo[:, :], in0=fcsum[0:B, :], in1=fbb_sb[:, :],
                    op=ALU.add)

                # AllGather the [8, 128] slices
                ag_in = dram.tile([B, OSL], dt.float32, name="ag_in")
                ag_out = dram.tile([NCORES * B, OSL], dt.float32, name="ag_out")
                nc.sync.dma_start(out=ag_in[:, :], in_=fo[:, :])
                nc.gpsimd.collective_compute(
                    "AllGather",
                    ALU.bypass,
                    replica_groups=[list(range(NCORES))],
                    ins=[ag_in[:, :]],
                    outs=[ag_out[:, :]],
                )
                # reassemble [8, 1024], then final l2norm computed in place
                fin = fpool.tile([B, OUT], dt.float32, name="fin")
                agv = ag_out.rearrange("(c b) o -> b c o", b=B)
                nc.sync.dma_start(
                    out=fin.rearrange("b (c o) -> b c o", c=NCORES),
                    in_=agv[:, :, :],
                )
                fsc = fpool.tile([B, OUT], dt.float32, name="fsc")
                ssf = fpool.tile([B, 1], dt.float32, name="ssf")
                lnf = fpool.tile([B, 1], dt.float32, name="lnf")
                rnf = fpool.tile([B, 1], dt.float32, name="rnf")
                nc.vector.scalar_tensor_tensor(
                    out=fsc[:, :], in0=fin[:, :], scalar=1.0,
                    in1=fin[:, :], op0=ALU.mult, op1=ALU.mult,
                    accum_out=ssf[:, 0:1])
                nc.vector.tensor_scalar_max(ssf[:, :], ssf[:, :], 1e-24)
                nc.scalar.activation(lnf[:, :], ssf[:, :], AF.Ln)
                nc.scalar.activation(rnf[:, :], lnf[:, :], AF.Exp, scale=-0.5)
                fout = fpool.tile([B, OUT], dt.float32, name="fout")
                nc.vector.tensor_scalar(
                    out=fout[:, :], in0=fin[:, :],
                    scalar1=rnf[:, 0:1], scalar2=None, op0=ALU.mult)
                nc.sync.dma_start(out=out_d[:, :], in_=fout[:, :])

            vpool_ctx.__exit__(None, None, None)

    # Force every activation onto the one table set that holds Exp+Ln
    # (+Copy/Identity) together -- the default per-function choice thrashes
    # ACT_TABLE_LOADs (~1.3us each) between exp_and_others / natural_log.
    import types
    import bass_rust as _bass_rust
    from concourse.hw_specs import get_activation_tables

    def _act_tables_one_set(self):
        has_activation = any(
            isinstance(i, mybir.InstActivation)
            for b in self.main_func.blocks
            for i in b.instructions
        )
        if not has_activation:
            return
        tables = get_activation_tables(self.m.arch)
        pref = "natural_log_exp_and_others"
        mod = [(k, (v if k == pref else set())) for k, v in tables.items()]
        _bass_rust.insert_act_table_loads(self, mod)

    nc.insert_act_table_loads = types.MethodType(_act_tables_one_set, nc)

    nc.compile()
    return nc


# ----------------------------------------------------------------------------
# Host-side input assembly per core
# ----------------------------------------------------------------------------

def _make_in_maps(feat, batch_ids, conv_w, conv_b, centroids, fc_w, fc_b):
    core_x, core_m, Ts = _plan(feat, batch_ids)

    wt = np.ascontiguousarray(conv_w.T / 8.0).astype(FP8)      # [256, 64]
    # conv_b split into 3 bf16 rows (exact to ~1e-4 abs), tiled G times
    b_hi = conv_b.astype(BF16)
    r1 = conv_b - b_hi.astype(np.float32)
    b_mid = r1.astype(BF16)
    r2 = r1 - b_mid.astype(np.float32)
    b_lo = r2.astype(BF16)
    b3 = np.stack([b_hi, b_mid, b_lo], 0)                       # [3, 64]
    b3t = np.tile(b3, (1, G))                                   # [3, 512]
    ones3 = np.ones((3, P), np.float32).astype(BF16)

    cent2 = np.concatenate([centroids, centroids], 0).astype(BF16)  # [128, 256]
    ident = np.eye(P, dtype=np.float32).astype(BF16)
    sel = np.zeros((P, B), np.float32)
    for g in range(4):
        for b in range(B):
            sel[32 * g + b, b] = 1.0

    OSL = OUT // NCORES
    in_maps = []
    for i in range(NCORES):
        cx = core_x[i]
        cm = core_m[i]
        nt = cx.shape[0] // P
        featN = np.empty((P, nt, C + 1), dtype=BF16)
        featN[:, :, 0:C] = cx.reshape(nt, P, C).transpose(1, 0, 2).astype(BF16)
        featN[:, :, C] = cm.reshape(nt, P).T.astype(BF16)
        featT = np.ascontiguousarray(cx.T * 8.0).astype(FP8)
        # fc slice, negated (vlad computed negated), chunk-major pre-swizzle:
        # fwt_sb[p, j*128+o] = -fc_w[o_base+o, j*128+p]
        fsl = -fc_w[i * OSL:(i + 1) * OSL]                      # [128, 16384]
        fsw = np.ascontiguousarray(
            fsl.reshape(OSL, K * C // P, P).transpose(2, 1, 0).reshape(P, K * C)
        ).astype(BF16)
        fbb = np.broadcast_to(fc_b[i * OSL:(i + 1) * OSL].astype(np.float32),
                              (B, OSL)).copy()
        in_maps.append({
            "featN": featN,
            "featT": featT,
            "wt": wt,
            "b3": b3t,
            "ones3": ones3,
            "cent2": cent2,
            "fwt": fsw,
            "fbb": fbb,
            "ident": ident,
            "sel": sel,
        })
    return in_maps, Ts


def _ensure_profile_hook():
    """The agent image's `antenv` lacks `axon_hooks`; synthesize it so
    run_bass_kernel_spmd(trace=True) can reach the NTFF profiler."""
    import sys
    import types
    try:
        from antenv.axon_hooks import get_axon_ntff_profile_hook  # noqa: F401
        return True
    except ImportError:
        pass
    try:
        from trn_agent_boot.trn_boot import _ntff_profile_via_ctypes
        hook = _ntff_profile_via_ctypes("/opt/axon/libaxon_pjrt.so")
        if hook is None:
            return False
        mod = types.ModuleType("antenv.axon_hooks")
        mod._hook = hook
        mod.get_axon_ntff_profile_hook = lambda: mod._hook
        mod.set_axon_ntff_profile_hook = lambda h: setattr(mod, "_hook", h)
        import antenv
        antenv.axon_hooks = mod
        sys.modules["antenv.axon_hooks"] = mod
        return True
    except Exception:
        return False


def kernel(feat, batch_ids, centroids, conv_w, conv_b, fc_w, fc_b, batch_size):
    from concourse.bass_utils import run_bass_kernel_spmd

    feat = np.asarray(feat, dtype=np.float32)
    batch_ids = np.asarray(batch_ids, dtype=np.int32)
    centroids = np.asarray(centroids, dtype=np.float32)
    conv_w = np.asarray(conv_w, dtype=np.float32)
    conv_b = np.asarray(conv_b, dtype=np.float32)
    fc_w = np.asarray(fc_w, dtype=np.float32)
    fc_b = np.asarray(fc_b, dtype=np.float32)

    in_maps, Ts = _make_in_maps(
        feat, batch_ids, conv_w, conv_b, centroids, fc_w, fc_b)

    key = tuple(Ts)
    if key not in _compiled_cache:
        _compiled_cache[key] = _build_nc(Ts)
    nc = _compiled_cache[key]

    global LAST_RESULT
    do_trace = PROFILE and _ensure_profile_hook()
    res = run_bass_kernel_spmd(
        nc, in_maps, core_ids=list(range(NCORES)), trace=do_trace)
    LAST_RESULT = res
    return np.asarray(res.results[0]["out"], dtype=np.float32)
